# revision 25
# baseline (speedup 1.0000x reference)
"""Trainium2 Bass kernel for nn_DMlp_46823733461564 (dense_mlp).

Computes: token-grid 3x3 masked-neighborhood gather (pixel-shuffle +
reflection-pad + masked unfold, algebraically reduced to a channel-
permuted shifted gather) followed by fc1(1600->1024) + exact GELU +
fc2(1024->576).

Sharding: data-parallel over (batch, image-half) -> 8 cores, 8192 tokens
each; fc weights replicated. The gather runs on-device as strided DMAs
from a host-prepared reflection-extended channel-major image; matmuls run
in bf16 on the PE (fp32 PSUM accumulation), ~3e-3 relative error.
"""
import os
import sys

import numpy as np

_TRN_REPO = "/opt/trn_rl_repo"
if _TRN_REPO not in sys.path:
    sys.path.insert(0, _TRN_REPO)

B, HIMG, WIMG = 4, 128, 128
C = 64
L = 576           # C * 9
NTOK = HIMG * WIMG
HID = 1024
OUTF = 576
INF = 1600        # C * 25
N_CORES = 8
ROWS_PER_CORE = HIMG // 2          # 64 token rows
TOK_PER_CORE = ROWS_PER_CORE * WIMG  # 8192
TILE_ROWS = 4                      # image rows per token tile
TT = TILE_ROWS * WIMG              # 512 tokens per tile
N_TILES = ROWS_PER_CORE // TILE_ROWS  # 16
KC = 13                            # ceil(25/2) K-chunks of (up to) 128

_MASK = np.array([
    [1, 0, 0, 1, 0, 0, 1],
    [0, 1, 0, 1, 0, 1, 0],
    [0, 0, 1, 1, 1, 0, 0],
    [1, 1, 1, 1, 1, 1, 1],
    [0, 0, 1, 1, 1, 0, 0],
    [0, 1, 0, 1, 0, 1, 0],
    [1, 0, 0, 1, 0, 0, 1]], dtype=bool)
MASK_POS = [(i, j) for i in range(7) for j in range(7) if _MASK[i, j]]


def _dmap(d):
    if d <= 1:
        return -1, d + 1
    if d <= 4:
        return 0, d - 2
    return 1, d - 5


KPOS = []
for (_di, _dj) in MASK_POS:
    _dh, _r1 = _dmap(_di)
    _dw, _r2 = _dmap(_dj)
    KPOS.append((_dh, _dw, _r1 * 3 + _r2))


def _swap_map(a, b, which):
    ch = np.arange(L)
    c, rem = ch // 9, ch % 9
    r1, r2 = rem // 3, rem % 3
    r = r1 if which == 0 else r2
    rs = np.where(r == a, b, np.where(r == b, a, r))
    if which == 0:
        return c * 9 + rs * 3 + r2
    return c * 9 + r1 * 3 + rs


def _build_xe(x):
    """x: (B, NTOK, L) -> xe: (B, L, HIMG+2, WIMG+2) reflection-extended,
    channel-permuted borders."""
    xt = np.ascontiguousarray(x.transpose(0, 2, 1)).reshape(B, L, HIMG, WIMG)
    xe = np.empty((B, L, HIMG + 2, WIMG + 2), dtype=np.float32)
    xe[:, :, 1:-1, 1:-1] = xt
    xe[:, :, 0, 1:-1] = xt[:, _swap_map(1, 2, 0), 0, :]
    xe[:, :, -1, 1:-1] = xt[:, _swap_map(0, 1, 0), -1, :]
    xe[:, :, :, 0] = np.take(xe[:, :, :, 1], _swap_map(1, 2, 1), axis=1)
    xe[:, :, :, -1] = np.take(xe[:, :, :, -2], _swap_map(0, 1, 1), axis=1)
    return xe


_NC_CACHE = {}

MODE = os.environ.get("KERNEL_DTYPE", "bf16")  # "f32r" | "bf16"

_WS_COUNTER = [0]


def _split_waits(nc, limit=1):
    """walrus in this toolchain accepts only ONE sync wait per instruction;
    move excess waits onto same-engine NoOps inserted just before (engine
    program order makes this equivalent)."""
    import concourse.mybir as mybir

    def noop(engine, waits):
        _WS_COUNTER[0] += 1
        return mybir.InstNoOp(
            name=f"WS-{_WS_COUNTER[0]}",
            sync_info=mybir.SyncInfo(on_wait=list(waits), on_update=[]),
            bass_nofuse=True,
            engine=engine,
        )

    for fn in nc.m.functions:
        for blk in fn.blocks:
            new_insts = []
            for inst in blk.instructions:
                si = getattr(inst, "sync_info", None)
                waits = list(si.on_wait) if si and si.on_wait else []
                if len(waits) > limit:
                    excess = waits[: len(waits) - limit]
                    si.on_wait = waits[len(waits) - limit:]
                    while excess:
                        new_insts.append(noop(inst.engine, excess[:limit]))
                        excess = excess[limit:]
                new_insts.append(inst)
            blk.instructions = new_insts


def _build_bass():
    if "nc" in _NC_CACHE:
        return _NC_CACHE["nc"]
    import concourse.bass as bass
    import concourse.mybir as mybir
    from concourse.tile import TileContext

    f32 = mybir.dt.float32
    mm_dt = mybir.dt.float32r if MODE == "f32r" else mybir.dt.bfloat16
    AF = mybir.ActivationFunctionType
    Alu = mybir.AluOpType

    nc = bass.Bass("TRN2", target_bir_lowering=False, debug=False)
    xe = nc.dram_tensor("xe", (L, ROWS_PER_CORE + 2, WIMG + 2), mm_dt,
                        kind="ExternalInput")
    w1p = nc.dram_tensor("w1p", (INF, HID), mm_dt, kind="ExternalInput")
    w2t = nc.dram_tensor("w2t", (HID, OUTF), mm_dt, kind="ExternalInput")
    b1rs = nc.dram_tensor("b1rs", (128, HID // 128), f32, kind="ExternalInput")
    b2bc = nc.dram_tensor("b2bc", (128, OUTF), f32, kind="ExternalInput")
    out = nc.dram_tensor("out", (TOK_PER_CORE, OUTF), f32,
                         kind="ExternalOutput")

    # view of xe with the q sub-pixel index split out: [q, c, rows, cols]
    xe_q = xe.rearrange("(c q) h w -> q c h w", q=9)

    with TileContext(nc) as tc:
        with (
            tc.tile_pool(name="wpool", bufs=1) as wpool,
            tc.tile_pool(name="fpool", bufs=3) as fpool,
            tc.tile_pool(name="hpool", bufs=2) as hpool,
            tc.tile_pool(name="opool", bufs=3) as opool,
            tc.tile_pool(name="ps1", bufs=2, space="PSUM") as ps1,
            tc.tile_pool(name="ps2", bufs=2, space="PSUM") as ps2,
        ):
            pack64 = os.environ.get("KERNEL_PACK64", "1") == "1"
            # --- PE warmup: dependency-free matmuls fill the initial DMA
            # wait and push the HAM clock gate to 8/8 before real work ---
            n_warm = int(os.environ.get("KERNEL_WARMUP", "160"))
            warm_n = int(os.environ.get("KERNEL_WARMUP_N", "128"))
            if n_warm:
                warm = wpool.tile([128, 512], mm_dt, tag="warm")
                nc.vector.memset(warm[:, :], 0.0)
                wps = ps2.tile([128, 512], f32, tag="poa")
                for _ in range(n_warm):
                    nc.tensor.matmul(wps[:, 0:warm_n], warm[:, 0:128],
                                     warm[:, 0:warm_n], start=True, stop=True)
            # --- replicated weights, loaded once.  fc1 weights go on the
            # scalar HWDGE queue so they overlap the tile-0 feat gathers
            # (which go on sync); fc2 weights/biases are needed later ---
            w1sb = []
            for j in range(KC):
                kr = min(128, INF - j * 128)
                if kr < 128 and pack64:
                    # duplicate the K=64 tail into partitions 64:128 so the
                    # packed row-group matmul can read lhsT from there
                    t = wpool.tile([128, HID], mm_dt, tag=f"w1_{j}")
                    nc.scalar.dma_start(out=t[0:kr, :], in_=w1p[j * 128 :, :])
                    nc.scalar.dma_start(out=t[kr:128, :], in_=w1p[j * 128 :, :])
                else:
                    t = wpool.tile([kr, HID], mm_dt, tag=f"w1_{j}")
                    nc.scalar.dma_start(out=t[:, :], in_=w1p[j * 128 : j * 128 + kr, :])
                w1sb.append(t)
            b1t = wpool.tile([128, HID // 128], f32, tag="b1")
            nc.scalar.dma_start(out=b1t[:, :], in_=b1rs[:, :])
            # fc2 weights + b2 are loaded after tile-0's gather is queued
            # (they aren't needed until ~30us in)
            w2sb = []
            b2t = None

            def emit_fc2(hts, r0, tt):
                # --- fc2: out[tok, :] = h.T @ w2t + b2 ---
                # N split 288+288 so both matmuls stream well past the
                # (hidden) LDWEIGHTS; each [128, 288] psum is one bank.
                NH = OUTF // 2
                for s in range(tt // 128):
                    poa = ps2.tile([128, NH], f32, tag="poa")
                    pob = ps2.tile([128, NH], f32, tag="pob")
                    for j in range(HID // 128):
                        nc.tensor.matmul(
                            poa[:, :],
                            hts[j][:, s * 128 : (s + 1) * 128],
                            w2sb[j][:, 0:NH],
                            start=(j == 0), stop=(j == HID // 128 - 1),
                        )
                        nc.tensor.matmul(
                            pob[:, :],
                            hts[j][:, s * 128 : (s + 1) * 128],
                            w2sb[j][:, NH:OUTF],
                            start=(j == 0), stop=(j == HID // 128 - 1),
                        )
                    ot = opool.tile([128, OUTF], f32, tag="o")
                    nc.vector.tensor_tensor(
                        out=ot[:, 0:NH], in0=poa[:, :], in1=b2t[:, 0:NH],
                        op=Alu.add)
                    nc.vector.tensor_tensor(
                        out=ot[:, NH:OUTF], in0=pob[:, :], in1=b2t[:, NH:OUTF],
                        op=Alu.add)
                    tok0 = (r0 * WIMG) + s * 128
                    nc.sync.dma_start(out=out[tok0 : tok0 + 128, :], in_=ot[:, :])

            # tile list: 4-row tiles, except the first tile (split so
            # compute starts after less DMA data) and the last tile (split
            # so the serial fc2 tail after the final fc1 is shorter)
            half = TILE_ROWS // 2
            tiles = [(0, half), (half, half)]
            tiles += [(i * TILE_ROWS, TILE_ROWS) for i in range(1, N_TILES - 1)]
            tiles += [((N_TILES - 1) * TILE_ROWS, half),
                      ((N_TILES - 1) * TILE_ROWS + half, half)]

            prev = None  # (hts, r0, tt) of the previous tile: fc2 runs one
            # tile behind fc1 so the PE never waits on the GELU latency
            for t_i, (r0, nrows) in enumerate(tiles):
                tt = nrows * WIMG
                # --- gather featT tile: rows k*64+c, k-pairs per 128-chunk ---
                fts = []
                for j in range(KC):
                    nk = 2 if 2 * j + 1 < 25 else 1
                    kr = 128 if (nk == 2 or pack64) else 64
                    ft = fpool.tile([kr, tt], mm_dt, tag=f"f{j}")
                    fts.append(ft)
                    for half in range(kr // 64):
                        k = min(2 * j + half, 24)
                        dh, dw, q = KPOS[k]
                        src = xe_q[q, :, 1 + r0 + dh : 1 + r0 + dh + nrows,
                                   1 + dw : 1 + dw + WIMG]
                        dst = ft[half * 64 : (half + 1) * 64, :].rearrange(
                            "p (r w) -> p r w", r=nrows)
                        # spread issue cost across both HWDGE rings; tile 0
                        # stays on sync (scalar ring is busy with weights)
                        eng = nc.sync if (t_i == 0 or j % 2 == 0) else nc.scalar
                        eng.dma_start(out=dst, in_=src)
                if t_i == 0:
                    for j2 in range(HID // 128):
                        t = wpool.tile([128, OUTF], mm_dt, tag=f"w2_{j2}")
                        nc.scalar.dma_start(
                            out=t[:, :], in_=w2t[j2 * 128 : (j2 + 1) * 128, :])
                        w2sb.append(t)
                    b2t = wpool.tile([128, OUTF], f32, tag="b2")
                    nc.scalar.dma_start(out=b2t[:, :], in_=b2bc[:, :])
                # --- fc1 + GELU: h[m] = gelu(w1p[:,m].T @ featT + b1) ---
                hts = []
                if not pack64:
                    for m in range(HID // 128):
                        ps = ps1.tile([128, tt], f32)
                        for j in range(KC):
                            nc.tensor.matmul(
                                ps[:, :],
                                w1sb[j][:, m * 128 : (m + 1) * 128],
                                fts[j][:, :],
                                start=(j == 0), stop=(j == KC - 1),
                            )
                        ht = hpool.tile([128, tt], mm_dt, tag=f"h{m}")
                        nc.scalar.activation(ht[:, :], ps[:, :], AF.Gelu,
                                             bias=b1t[:, m : m + 1], scale=1.0)
                        hts.append(ht)
                else:
                    # chunk 12 (K=64) packed: m-pairs run their K=64 matmuls
                    # concurrently on PE row groups (0,0)/(64,0)
                    for mp in range(HID // 256):
                        m0, m1 = 2 * mp, 2 * mp + 1
                        psa = ps1.tile([128, tt], f32, tag="psa")
                        psb = ps1.tile([128, tt], f32, tag="psb")
                        for j in range(KC - 1):
                            nc.tensor.matmul(
                                psa[:, :], w1sb[j][:, m0 * 128:(m0 + 1) * 128],
                                fts[j][:, :], start=(j == 0), stop=False)
                            nc.tensor.matmul(
                                psb[:, :], w1sb[j][:, m1 * 128:(m1 + 1) * 128],
                                fts[j][:, :], start=(j == 0), stop=False)
                        nc.tensor.matmul(
                            psa[:, :], w1sb[KC - 1][0:64, m0 * 128:(m0 + 1) * 128],
                            fts[KC - 1][0:64, :], start=False, stop=True)
                        nc.tensor.matmul(
                            psb[:, :], w1sb[KC - 1][64:128, m1 * 128:(m1 + 1) * 128],
                            fts[KC - 1][64:128, :], start=False, stop=True)
                        for m, pst in ((m0, psa), (m1, psb)):
                            ht = hpool.tile([128, tt], mm_dt, tag=f"h{m}")
                            nc.scalar.activation(ht[:, :], pst[:, :], AF.Gelu,
                                                 bias=b1t[:, m : m + 1], scale=1.0)
                            hts.append(ht)
                if prev is not None:
                    emit_fc2(*prev)
                prev = (hts, r0, tt)
            emit_fc2(*prev)

    _split_waits(nc)
    _NC_CACHE["nc"] = nc
    return nc


def _host_prep(x, w1, b1, w2, b2):
    x = np.ascontiguousarray(np.asarray(x, dtype=np.float32))
    w1 = np.asarray(w1, dtype=np.float32)
    b1 = np.asarray(b1, dtype=np.float32)
    w2 = np.asarray(w2, dtype=np.float32)
    b2 = np.asarray(b2, dtype=np.float32)

    xe = _build_xe(x)
    w1t = np.ascontiguousarray(w1.T)  # (1600, 1024) rows c*25+k
    w1p = np.ascontiguousarray(
        w1t.reshape(C, 25, HID).transpose(1, 0, 2).reshape(INF, HID))
    w2t = np.ascontiguousarray(w2.T)
    b1rs = np.ascontiguousarray(b1.reshape(HID // 128, 128).T)
    b2bc = np.ascontiguousarray(np.broadcast_to(b2, (128, OUTF)))

    if MODE == "bf16":
        import ml_dtypes
        xe = xe.astype(ml_dtypes.bfloat16)
        w1p = w1p.astype(ml_dtypes.bfloat16)
        w2t = w2t.astype(ml_dtypes.bfloat16)

    in_maps = []
    for cid in range(N_CORES):
        b, half = cid // 2, cid % 2
        h0 = half * ROWS_PER_CORE
        xe_core = np.ascontiguousarray(xe[b, :, h0 : h0 + ROWS_PER_CORE + 2, :])
        in_maps.append({
            "xe": xe_core, "w1p": w1p, "w2t": w2t, "b1rs": b1rs, "b2bc": b2bc,
        })
    return in_maps


def _assemble(results):
    out = np.empty((B, NTOK, OUTF), dtype=np.float32)
    for cid in range(N_CORES):
        b, half = cid // 2, cid % 2
        t0 = half * TOK_PER_CORE
        out[b, t0 : t0 + TOK_PER_CORE, :] = results[cid]["out"]
    return out


def kernel(x, w1, b1, w2, b2, image_h, image_w):
    in_maps = _host_prep(x, w1, b1, w2, b2)
    nc = _build_bass()
    from concourse.bass_utils import run_bass_kernel_spmd
    res = run_bass_kernel_spmd(nc, in_maps, list(range(N_CORES)))
    return _assemble(res.results)


# revision 26
# speedup vs baseline: 1.0105x; 1.0105x over previous
"""Trainium2 Bass kernel for nn_DMlp_46823733461564 (dense_mlp).

Computes: token-grid 3x3 masked-neighborhood gather (pixel-shuffle +
reflection-pad + masked unfold, algebraically reduced to a channel-
permuted shifted gather) followed by fc1(1600->1024) + exact GELU +
fc2(1024->576).

Sharding: data-parallel over (batch, image-half) -> 8 cores, 8192 tokens
each; fc weights replicated. The gather runs on-device as strided DMAs
from a host-prepared reflection-extended channel-major image; matmuls run
in bf16 on the PE (fp32 PSUM accumulation), ~3e-3 relative error.
"""
import os
import sys

import numpy as np

_TRN_REPO = "/opt/trn_rl_repo"
if _TRN_REPO not in sys.path:
    sys.path.insert(0, _TRN_REPO)

B, HIMG, WIMG = 4, 128, 128
C = 64
L = 576           # C * 9
NTOK = HIMG * WIMG
HID = 1024
OUTF = 576
INF = 1600        # C * 25
N_CORES = 8
ROWS_PER_CORE = HIMG // 2          # 64 token rows
TOK_PER_CORE = ROWS_PER_CORE * WIMG  # 8192
TILE_ROWS = 4                      # image rows per token tile
TT = TILE_ROWS * WIMG              # 512 tokens per tile
N_TILES = ROWS_PER_CORE // TILE_ROWS  # 16
KC = 13                            # ceil(25/2) K-chunks of (up to) 128

_MASK = np.array([
    [1, 0, 0, 1, 0, 0, 1],
    [0, 1, 0, 1, 0, 1, 0],
    [0, 0, 1, 1, 1, 0, 0],
    [1, 1, 1, 1, 1, 1, 1],
    [0, 0, 1, 1, 1, 0, 0],
    [0, 1, 0, 1, 0, 1, 0],
    [1, 0, 0, 1, 0, 0, 1]], dtype=bool)
MASK_POS = [(i, j) for i in range(7) for j in range(7) if _MASK[i, j]]


def _dmap(d):
    if d <= 1:
        return -1, d + 1
    if d <= 4:
        return 0, d - 2
    return 1, d - 5


KPOS = []
for (_di, _dj) in MASK_POS:
    _dh, _r1 = _dmap(_di)
    _dw, _r2 = _dmap(_dj)
    KPOS.append((_dh, _dw, _r1 * 3 + _r2))


def _swap_map(a, b, which):
    ch = np.arange(L)
    c, rem = ch // 9, ch % 9
    r1, r2 = rem // 3, rem % 3
    r = r1 if which == 0 else r2
    rs = np.where(r == a, b, np.where(r == b, a, r))
    if which == 0:
        return c * 9 + rs * 3 + r2
    return c * 9 + r1 * 3 + rs


def _build_xe(x):
    """x: (B, NTOK, L) -> xe: (B, L, HIMG+2, WIMG+2) reflection-extended,
    channel-permuted borders."""
    xt = np.ascontiguousarray(x.transpose(0, 2, 1)).reshape(B, L, HIMG, WIMG)
    xe = np.empty((B, L, HIMG + 2, WIMG + 2), dtype=np.float32)
    xe[:, :, 1:-1, 1:-1] = xt
    xe[:, :, 0, 1:-1] = xt[:, _swap_map(1, 2, 0), 0, :]
    xe[:, :, -1, 1:-1] = xt[:, _swap_map(0, 1, 0), -1, :]
    xe[:, :, :, 0] = np.take(xe[:, :, :, 1], _swap_map(1, 2, 1), axis=1)
    xe[:, :, :, -1] = np.take(xe[:, :, :, -2], _swap_map(0, 1, 1), axis=1)
    return xe


_NC_CACHE = {}

MODE = os.environ.get("KERNEL_DTYPE", "bf16")  # "f32r" | "bf16"

_WS_COUNTER = [0]


def _split_waits(nc, limit=1):
    """walrus in this toolchain accepts only ONE sync wait per instruction;
    move excess waits onto same-engine NoOps inserted just before (engine
    program order makes this equivalent)."""
    import concourse.mybir as mybir

    def noop(engine, waits):
        _WS_COUNTER[0] += 1
        return mybir.InstNoOp(
            name=f"WS-{_WS_COUNTER[0]}",
            sync_info=mybir.SyncInfo(on_wait=list(waits), on_update=[]),
            bass_nofuse=True,
            engine=engine,
        )

    for fn in nc.m.functions:
        for blk in fn.blocks:
            new_insts = []
            for inst in blk.instructions:
                si = getattr(inst, "sync_info", None)
                waits = list(si.on_wait) if si and si.on_wait else []
                if len(waits) > limit:
                    excess = waits[: len(waits) - limit]
                    si.on_wait = waits[len(waits) - limit:]
                    while excess:
                        new_insts.append(noop(inst.engine, excess[:limit]))
                        excess = excess[limit:]
                new_insts.append(inst)
            blk.instructions = new_insts


def _build_bass():
    if "nc" in _NC_CACHE:
        return _NC_CACHE["nc"]
    import concourse.bass as bass
    import concourse.mybir as mybir
    from concourse.tile import TileContext

    f32 = mybir.dt.float32
    mm_dt = mybir.dt.float32r if MODE == "f32r" else mybir.dt.bfloat16
    AF = mybir.ActivationFunctionType
    Alu = mybir.AluOpType

    nc = bass.Bass("TRN2", target_bir_lowering=False, debug=False)
    xe = nc.dram_tensor("xe", (L, ROWS_PER_CORE + 2, WIMG + 2), mm_dt,
                        kind="ExternalInput")
    w1p = nc.dram_tensor("w1p", (INF, HID), mm_dt, kind="ExternalInput")
    w2t = nc.dram_tensor("w2t", (HID, OUTF), mm_dt, kind="ExternalInput")
    b1rs = nc.dram_tensor("b1rs", (128, HID // 128), f32, kind="ExternalInput")
    b2bc = nc.dram_tensor("b2bc", (128, OUTF), f32, kind="ExternalInput")
    out = nc.dram_tensor("out", (TOK_PER_CORE, OUTF), f32,
                         kind="ExternalOutput")

    # view of xe with the q sub-pixel index split out: [q, c, rows, cols]
    xe_q = xe.rearrange("(c q) h w -> q c h w", q=9)

    with TileContext(nc) as tc:
        with (
            tc.tile_pool(name="wpool", bufs=1) as wpool,
            tc.tile_pool(name="fpool", bufs=3) as fpool,
            tc.tile_pool(name="hpool", bufs=2) as hpool,
            tc.tile_pool(name="opool", bufs=3) as opool,
            tc.tile_pool(name="ps1", bufs=2, space="PSUM") as ps1,
            tc.tile_pool(name="ps2", bufs=2, space="PSUM") as ps2,
        ):
            pack64 = os.environ.get("KERNEL_PACK64", "1") == "1"
            # --- PE warmup: dependency-free matmuls fill the initial DMA
            # wait and push the HAM clock gate to 8/8 before real work ---
            n_warm = int(os.environ.get("KERNEL_WARMUP", "160"))
            warm_n = int(os.environ.get("KERNEL_WARMUP_N", "128"))
            if n_warm:
                warm = wpool.tile([128, 512], mm_dt, tag="warm")
                nc.vector.memset(warm[:, :], 0.0)
                wps = ps2.tile([128, 512], f32, tag="poa")
                for _ in range(n_warm):
                    nc.tensor.matmul(wps[:, 0:warm_n], warm[:, 0:128],
                                     warm[:, 0:warm_n], start=True, stop=True)
            # --- replicated weights, loaded once.  fc1 weights go on the
            # scalar HWDGE queue so they overlap the tile-0 feat gathers
            # (which go on sync); fc2 weights/biases are needed later ---
            w1sb = []
            for j in range(KC):
                kr = min(128, INF - j * 128)
                if kr < 128 and pack64:
                    # duplicate the K=64 tail into partitions 64:128 so the
                    # packed row-group matmul can read lhsT from there
                    t = wpool.tile([128, HID], mm_dt, tag=f"w1_{j}")
                    nc.scalar.dma_start(out=t[0:kr, :], in_=w1p[j * 128 :, :])
                    nc.scalar.dma_start(out=t[kr:128, :], in_=w1p[j * 128 :, :])
                else:
                    t = wpool.tile([kr, HID], mm_dt, tag=f"w1_{j}")
                    nc.scalar.dma_start(out=t[:, :], in_=w1p[j * 128 : j * 128 + kr, :])
                w1sb.append(t)
            b1t = wpool.tile([128, HID // 128], f32, tag="b1")
            nc.scalar.dma_start(out=b1t[:, :], in_=b1rs[:, :])
            # fc2 weights + b2 are loaded after tile-0's gather is queued
            # (they aren't needed until ~30us in)
            w2sb = []
            b2t = None

            def emit_fc2(hts, r0, tt):
                # --- fc2: out[tok, :] = h.T @ w2t + b2 ---
                # N split 288+288 so both matmuls stream well past the
                # (hidden) LDWEIGHTS; each [128, 288] psum is one bank.
                NH = OUTF // 2
                for s in range(tt // 128):
                    poa = ps2.tile([128, NH], f32, tag="poa")
                    pob = ps2.tile([128, NH], f32, tag="pob")
                    for j in range(HID // 128):
                        nc.tensor.matmul(
                            poa[:, :],
                            hts[j][:, s * 128 : (s + 1) * 128],
                            w2sb[j][:, 0:NH],
                            start=(j == 0), stop=(j == HID // 128 - 1),
                        )
                        nc.tensor.matmul(
                            pob[:, :],
                            hts[j][:, s * 128 : (s + 1) * 128],
                            w2sb[j][:, NH:OUTF],
                            start=(j == 0), stop=(j == HID // 128 - 1),
                        )
                    ot = opool.tile([128, OUTF], f32, tag="o")
                    nc.vector.tensor_tensor(
                        out=ot[:, 0:NH], in0=poa[:, :], in1=b2t[:, 0:NH],
                        op=Alu.add)
                    nc.vector.tensor_tensor(
                        out=ot[:, NH:OUTF], in0=pob[:, :], in1=b2t[:, NH:OUTF],
                        op=Alu.add)
                    tok0 = (r0 * WIMG) + s * 128
                    nc.sync.dma_start(out=out[tok0 : tok0 + 128, :], in_=ot[:, :])

            # tile list: 4-row tiles, with the last tile split in half so
            # the serial fc2 tail after the final fc1 is shorter
            tiles = [(i * TILE_ROWS, TILE_ROWS) for i in range(N_TILES - 1)]
            tiles += [((N_TILES - 1) * TILE_ROWS, TILE_ROWS // 2),
                      ((N_TILES - 1) * TILE_ROWS + TILE_ROWS // 2,
                       TILE_ROWS // 2)]

            prev = None  # (hts, r0, tt) of the previous tile: fc2 runs one
            # tile behind fc1 so the PE never waits on the GELU latency
            for t_i, (r0, nrows) in enumerate(tiles):
                tt = nrows * WIMG
                # --- gather featT tile: rows k*64+c, k-pairs per 128-chunk ---
                fts = []
                for j in range(KC):
                    nk = 2 if 2 * j + 1 < 25 else 1
                    kr = 128 if (nk == 2 or pack64) else 64
                    ft = fpool.tile([kr, tt], mm_dt, tag=f"f{j}")
                    fts.append(ft)
                    for half in range(kr // 64):
                        k = min(2 * j + half, 24)
                        dh, dw, q = KPOS[k]
                        src = xe_q[q, :, 1 + r0 + dh : 1 + r0 + dh + nrows,
                                   1 + dw : 1 + dw + WIMG]
                        dst = ft[half * 64 : (half + 1) * 64, :].rearrange(
                            "p (r w) -> p r w", r=nrows)
                        # spread issue cost across both HWDGE rings; tile 0
                        # stays on sync (scalar ring is busy with weights)
                        eng = nc.sync if (t_i == 0 or j % 2 == 0) else nc.scalar
                        eng.dma_start(out=dst, in_=src)
                if t_i == 0:
                    for j2 in range(HID // 128):
                        t = wpool.tile([128, OUTF], mm_dt, tag=f"w2_{j2}")
                        nc.scalar.dma_start(
                            out=t[:, :], in_=w2t[j2 * 128 : (j2 + 1) * 128, :])
                        w2sb.append(t)
                    b2t = wpool.tile([128, OUTF], f32, tag="b2")
                    nc.scalar.dma_start(out=b2t[:, :], in_=b2bc[:, :])
                # --- fc1 + GELU: h[m] = gelu(w1p[:,m].T @ featT + b1) ---
                hts = []
                if not pack64:
                    for m in range(HID // 128):
                        ps = ps1.tile([128, tt], f32)
                        for j in range(KC):
                            nc.tensor.matmul(
                                ps[:, :],
                                w1sb[j][:, m * 128 : (m + 1) * 128],
                                fts[j][:, :],
                                start=(j == 0), stop=(j == KC - 1),
                            )
                        ht = hpool.tile([128, tt], mm_dt, tag=f"h{m}")
                        nc.scalar.activation(ht[:, :], ps[:, :], AF.Gelu,
                                             bias=b1t[:, m : m + 1], scale=1.0)
                        hts.append(ht)
                else:
                    # chunk 12 (K=64) packed: m-pairs run their K=64 matmuls
                    # concurrently on PE row groups (0,0)/(64,0)
                    for mp in range(HID // 256):
                        m0, m1 = 2 * mp, 2 * mp + 1
                        psa = ps1.tile([128, tt], f32, tag="psa")
                        psb = ps1.tile([128, tt], f32, tag="psb")
                        for j in range(KC - 1):
                            nc.tensor.matmul(
                                psa[:, :], w1sb[j][:, m0 * 128:(m0 + 1) * 128],
                                fts[j][:, :], start=(j == 0), stop=False)
                            nc.tensor.matmul(
                                psb[:, :], w1sb[j][:, m1 * 128:(m1 + 1) * 128],
                                fts[j][:, :], start=(j == 0), stop=False)
                        nc.tensor.matmul(
                            psa[:, :], w1sb[KC - 1][0:64, m0 * 128:(m0 + 1) * 128],
                            fts[KC - 1][0:64, :], start=False, stop=True)
                        nc.tensor.matmul(
                            psb[:, :], w1sb[KC - 1][64:128, m1 * 128:(m1 + 1) * 128],
                            fts[KC - 1][64:128, :], start=False, stop=True)
                        for m, pst in ((m0, psa), (m1, psb)):
                            ht = hpool.tile([128, tt], mm_dt, tag=f"h{m}")
                            nc.scalar.activation(ht[:, :], pst[:, :], AF.Gelu,
                                                 bias=b1t[:, m : m + 1], scale=1.0)
                            hts.append(ht)
                if prev is not None:
                    emit_fc2(*prev)
                prev = (hts, r0, tt)
            emit_fc2(*prev)

    _split_waits(nc)
    _NC_CACHE["nc"] = nc
    return nc


def _host_prep(x, w1, b1, w2, b2):
    x = np.ascontiguousarray(np.asarray(x, dtype=np.float32))
    w1 = np.asarray(w1, dtype=np.float32)
    b1 = np.asarray(b1, dtype=np.float32)
    w2 = np.asarray(w2, dtype=np.float32)
    b2 = np.asarray(b2, dtype=np.float32)

    xe = _build_xe(x)
    w1t = np.ascontiguousarray(w1.T)  # (1600, 1024) rows c*25+k
    w1p = np.ascontiguousarray(
        w1t.reshape(C, 25, HID).transpose(1, 0, 2).reshape(INF, HID))
    w2t = np.ascontiguousarray(w2.T)
    b1rs = np.ascontiguousarray(b1.reshape(HID // 128, 128).T)
    b2bc = np.ascontiguousarray(np.broadcast_to(b2, (128, OUTF)))

    if MODE == "bf16":
        import ml_dtypes
        xe = xe.astype(ml_dtypes.bfloat16)
        w1p = w1p.astype(ml_dtypes.bfloat16)
        w2t = w2t.astype(ml_dtypes.bfloat16)

    in_maps = []
    for cid in range(N_CORES):
        b, half = cid // 2, cid % 2
        h0 = half * ROWS_PER_CORE
        xe_core = np.ascontiguousarray(xe[b, :, h0 : h0 + ROWS_PER_CORE + 2, :])
        in_maps.append({
            "xe": xe_core, "w1p": w1p, "w2t": w2t, "b1rs": b1rs, "b2bc": b2bc,
        })
    return in_maps


def _assemble(results):
    out = np.empty((B, NTOK, OUTF), dtype=np.float32)
    for cid in range(N_CORES):
        b, half = cid // 2, cid % 2
        t0 = half * TOK_PER_CORE
        out[b, t0 : t0 + TOK_PER_CORE, :] = results[cid]["out"]
    return out


def kernel(x, w1, b1, w2, b2, image_h, image_w):
    in_maps = _host_prep(x, w1, b1, w2, b2)
    nc = _build_bass()
    from concourse.bass_utils import run_bass_kernel_spmd
    res = run_bass_kernel_spmd(nc, in_maps, list(range(N_CORES)))
    return _assemble(res.results)
